# revision 3
# baseline (speedup 1.0000x reference)
"""Causal self-attention (B=4, T=2048, C=1024, H=16) on 8 NeuronCores — v2.

Same sharding/layout as v1 (core c: batch c//2, head-half c%2; Megatron
column/row parallel; bf16 matmuls, fp32 PSUM; host fp32 sum of the two
per-batch partials).

v2 restructures the schedule around measured engine rates:
  PE   ~296-338 ns per 512-col MM (dual 64-row tiles stream concurrently),
  ACT  (N+352)/1.2 ns + ~150 dispatch (exp, dtype-independent, no fast mode),
  DVE  1x for f32/PSUM tensor_tensor, 2x/4x for bf16 tensor_scalar/copy.
The v1 per-j chain scores->exp->AV made phase 2 ACT-bound (~163us serial).
v2 changes:
  * ACT does ONLY the phase-2 exp. q/k bias evacuations moved to DVE
    tensor_scalar (per-partition bias column), v/p3 evacuations stay DVE.
  * Phase 2 per (pair, qc): scores pass (64-row dual tiles) with "filler"
    PE quanta (phase-1/phase-3 matmul groups) interleaved after every odd
    j, so the PE works while ACT streams exp; then a K=128 single-tile AV
    pass accumulating both key halves into ONE psy tile (kills the v1
    psyA/psyB DVE merge), then normalization (reciprocal_approx_fast +
    gpsimd broadcast + 2 muls).
  * PSUM: scores 2x[128,1024] (4 banks) + p1/p3 groups 2x[128,512]
    (2 banks) + psy [65,1024] (2 banks) = 8 banks; the filler groups have
    their own banks so they never compete with the scores/exp pipeline.
  * Filler supply: p1(tch=qc+1) groups + p3 groups for chunks already
    normalized feed waves qc=0..3; the p3 tail and next iteration's
    p1(tch=0) fill the inter-iteration boundary.
"""

import sys
import types

import numpy as np
from contextlib import ExitStack
from collections import deque

import ml_dtypes

import concourse.bass as bass
import concourse.mybir as mybir
import concourse.tile as tile
from concourse import bacc
from concourse.bass_utils import run_bass_kernel_spmd

try:  # pragma: no cover
    import antenv.axon_hooks  # noqa: F401
except ImportError:  # pragma: no cover
    import antenv

    _stub = types.ModuleType("antenv.axon_hooks")
    _stub.get_axon_ntff_profile_hook = lambda: None
    sys.modules["antenv.axon_hooks"] = _stub
    antenv.axon_hooks = _stub

F32 = mybir.dt.float32
BF16 = mybir.dt.bfloat16
EXP = mybir.ActivationFunctionType.Exp
NP_BF16 = ml_dtypes.bfloat16

B, T, C, H = 4, 2048, 1024, 16
HD = C // H              # 64 head dim
N_CORES = 8
HPC = H // 2             # 8 heads per core
MPC = C // 2             # 512 channels per core
MT = MPC // 128          # 4 m-tiles per core
CT = C // 128            # 8 contraction tiles
TC = T // 512            # 4 t-chunks
TT = T // 128            # 16 t-tiles
SCALE = float(1.0 / np.sqrt(HD))
W = 512                  # query chunk width

_CACHE = {}


def _build(bench_loops=None):
    import contextlib

    nc = bacc.Bacc()
    xT = nc.declare_dram_parameter("xT", [C, T], BF16, isOutput=False)
    wqT = nc.declare_dram_parameter("wqT", [C, MPC], BF16, isOutput=False)
    wkT = nc.declare_dram_parameter("wkT", [C, MPC], BF16, isOutput=False)
    wvT = nc.declare_dram_parameter("wvT", [C, MPC], BF16, isOutput=False)
    wpT = nc.declare_dram_parameter("wpT", [MPC, C], BF16, isOutput=False)
    bqc = nc.declare_dram_parameter("bqc", [128, MT], F32, isOutput=False)
    bkc = nc.declare_dram_parameter("bkc", [128, MT], F32, isOutput=False)
    bv = nc.declare_dram_parameter("bv", [1, MPC], F32, isOutput=False)
    bp = nc.declare_dram_parameter("bp", [1, C], F32, isOutput=False)
    mask01d = nc.declare_dram_parameter("mask01", [128, 128], BF16, isOutput=False)
    outp = nc.declare_dram_parameter("out", [T, C], BF16, isOutput=True)

    with tile.TileContext(nc) as tc:
        with ExitStack() as ctx:
            persist = ctx.enter_context(tc.tile_pool(name="persist", bufs=1))
            pool_P = ctx.enter_context(tc.tile_pool(name="pool_P", bufs=1))
            # PSUM budget (8 banks): scores 2x[128,1024]=4, p1/p3 groups
            # 2x[128,512]=2, psy [65,1024]=2.
            ps_sc = ctx.enter_context(tc.tile_pool(name="ps_sc", bufs=2, space="PSUM"))
            ps_mm = ctx.enter_context(tc.tile_pool(name="ps_mm", bufs=2, space="PSUM"))
            ps_y = ctx.enter_context(tc.tile_pool(name="ps_y", bufs=1, space="PSUM"))

            # ---- constants / small tensors ----
            mask01_sb = persist.tile([128, 128], BF16, name="mask01_sb")
            nc.sync.dma_start(out=mask01_sb, in_=mask01d[:, :])
            bqc_sb = persist.tile([128, MT], F32, name="bqc_sb")
            nc.sync.dma_start(out=bqc_sb, in_=bqc[:, :])
            bkc_sb = persist.tile([128, MT], F32, name="bkc_sb")
            nc.sync.dma_start(out=bkc_sb, in_=bkc[:, :])
            bv_sb = persist.tile([1, MPC], F32, name="bv_sb")
            nc.sync.dma_start(out=bv_sb, in_=bv[:, :])
            bp_sb = persist.tile([1, C], F32, name="bp_sb")
            nc.sync.dma_start(out=bp_sb, in_=bp[:, :])
            ones8 = persist.tile([128, 8], BF16, name="ones8")
            nc.vector.memset(ones8, 1.0)
            bv_bc = persist.tile([128, MPC], F32, name="bv_bc")
            nc.gpsimd.partition_broadcast(bv_bc, bv_sb)
            bp_bc = persist.tile([128, C], F32, name="bp_bc")
            nc.gpsimd.partition_broadcast(bp_bc, bp_sb)

            # ---- persistent activations ----
            qT_sb = [persist.tile([128, T], BF16, name=f"qT{m}") for m in range(MT)]
            kT_sb = [persist.tile([128, T], BF16, name=f"kT{m}") for m in range(MT)]
            vAug = [persist.tile([128, HPC * (HD + 1)], BF16, name=f"vAug{t}") for t in range(TT)]
            yT_sb = [persist.tile([128, T], BF16, name=f"yT{m}") for m in range(MT)]

            # constant ones-column of vAug, written once
            for t_ in range(TT):
                va = vAug[t_].rearrange("p (h w) -> p h w", w=HD + 1)
                nc.vector.tensor_copy(va[:, :, HD], ones8)

            # ---- weights in SBUF (loaded once) ----
            pool_w = ctx.enter_context(tc.tile_pool(name="pool_w", bufs=1))
            wq_t = [pool_w.tile([128, MPC], BF16, name=f"wq{c}") for c in range(CT)]
            wk_t = [pool_w.tile([128, MPC], BF16, name=f"wk{c}") for c in range(CT)]
            wv_t = [pool_w.tile([128, MPC], BF16, name=f"wv{c}") for c in range(CT)]
            for c in range(CT):
                nc.sync.dma_start(out=wq_t[c], in_=wqT[c * 128:(c + 1) * 128, :])
                nc.sync.dma_start(out=wk_t[c], in_=wkT[c * 128:(c + 1) * 128, :])
                nc.sync.dma_start(out=wv_t[c], in_=wvT[c * 128:(c + 1) * 128, :])
            wp_t = [pool_w.tile([128, C], BF16, name=f"wp{m}") for m in range(MT)]
            for m in range(MT):
                nc.sync.dma_start(out=wp_t[m], in_=wpT[m * 128:(m + 1) * 128, :])

            pool_xs = ctx.enter_context(tc.tile_pool(name="pool_xs", bufs=1))
            pool_n = ctx.enter_context(tc.tile_pool(name="pool_n", bufs=2))
            pool_o = ctx.enter_context(tc.tile_pool(name="pool_o", bufs=1))

            ADD = mybir.AluOpType.add

            # ============ phase-1 quanta (one matmul group each) ============
            def p1_quanta(tch):
                """12 closures: 4 q-groups, 4 k-groups, 4 v-groups for t-chunk
                tch. The first closure also issues the xs DMAs."""
                t0 = tch * 512
                xs = []

                def dma():
                    for c in range(CT):
                        x_ = pool_xs.tile([128, 512], BF16, name=f"xs_{tch}_{c}",
                                          tag="xs", bufs=16)
                        nc.sync.dma_start(out=x_, in_=xT[c * 128:(c + 1) * 128, t0:t0 + 512])
                        xs.append(x_)

                def qk_group(wt, bias_col, dst, m, first):
                    def run():
                        if first:
                            dma()
                        ps = ps_mm.tile([128, 512], F32, name=f"p1_{tch}_{id(wt)}_{m}",
                                        tag="psmm")
                        for c in range(CT):
                            nc.tensor.matmul(ps, wt[c][:, m * 128:(m + 1) * 128], xs[c],
                                             start=(c == 0), stop=(c == CT - 1))
                        nc.vector.tensor_scalar(out=dst[m][:, t0:t0 + 512], in0=ps,
                                                scalar1=bias_col[:, m:m + 1],
                                                scalar2=None, op0=ADD)
                    return run

                def v_group(tt):
                    def run():
                        tg = tch * 4 + tt
                        ps = ps_mm.tile([128, MPC], F32, name=f"psv_{tg}", tag="psmm")
                        for c in range(CT):
                            nc.tensor.matmul(ps, xs[c][:, tt * 128:(tt + 1) * 128], wv_t[c],
                                             start=(c == 0), stop=(c == CT - 1))
                        va = vAug[tg].rearrange("p (h w) -> p h w", w=HD + 1)
                        nc.vector.tensor_add(va[:, :, 0:HD],
                                             ps.rearrange("p (h w) -> p h w", w=HD),
                                             bv_bc.rearrange("p (h w) -> p h w", w=HD))
                    return run

                out = []
                for m in range(MT):
                    out.append(qk_group(wq_t, bqc_sb, qT_sb, m, first=(m == 0)))
                for m in range(MT):
                    out.append(qk_group(wk_t, bkc_sb, kT_sb, m, first=False))
                for tt in range(4):
                    out.append(v_group(tt))
                return out

            # ============ phase-3 quanta ============
            def p3_quantum(tt, nch):
                def run():
                    n0 = nch * 512
                    ps = ps_mm.tile([128, 512], F32, name=f"pso_{tt}_{nch}", tag="psmm")
                    for m in range(MT):
                        nc.tensor.matmul(ps, yT_sb[m][:, tt * 128:(tt + 1) * 128],
                                         wp_t[m][:, n0:n0 + 512],
                                         start=(m == 0), stop=(m == MT - 1))
                    o_sb = pool_o.tile([128, 512], BF16, name=f"o_{tt}_{nch}", tag="o", bufs=3)
                    nc.vector.tensor_add(o_sb, ps, bp_bc[:, n0:n0 + 512])
                    nc.sync.dma_start(out=outp[tt * 128:(tt + 1) * 128, n0:n0 + 512], in_=o_sb)
                return run

            def pop_filler(fq):
                """Emit one filler, preferring deadline-bound p1 quanta over
                p3 quanta that rolled over from earlier waves."""
                for i, (kind, q) in enumerate(fq):
                    if kind[0] == "p1":
                        del fq[i]
                        q()
                        return
                fq.popleft()[1]()

            # ============ phase-2 passes ============
            def scores_pass(p, qc, fq, npop):
                """64-row dual-tile score MMs + exp ACT + diagonal mask, with
                up to npop filler quanta spread over the odd j's. Returns P
                tiles."""
                hE, hO = 2 * p, 2 * p + 1
                kT_h, qT_h = kT_sb[p], qT_sb[p]
                q0 = qc * W
                jmax = 4 * qc + 3
                odd = list(range(1, jmax + 1, 2))
                pop_js = {odd[min(len(odd) - 1, (i + 1) * len(odd) // (npop + 1))]
                          for i in range(npop)}
                Ps = []
                for j in range(jmax + 1):
                    lo = max(0, 128 * j - q0)
                    pss = ps_sc.tile([128, 1024], F32, name=f"pss_{p}_{qc}_{j}", tag="pssc")
                    for half, co in ((0, 0), (64, W)):
                        nc.tensor.matmul(
                            pss[:, co + lo:co + W],
                            kT_h[half:half + HD, j * 128:(j + 1) * 128],
                            qT_h[half:half + HD, q0 + lo:q0 + W],
                            start=True, stop=True, tile_position=(half, 0))
                    P = pool_P.tile([128, 1024], BF16, name=f"P_{p}_{qc}_{j}",
                                    tag="P", bufs=18)
                    pv = pss.rearrange("p (g w) -> p g w", w=W)
                    Pv = P.rearrange("p (g w) -> p g w", w=W)
                    nc.scalar.activation(out=Pv[:, :, lo:W], in_=pv[:, :, lo:W],
                                         func=EXP, scale=SCALE)
                    if 128 * j >= q0:  # diagonal block: in-tile causal mask
                        for co in (0, W):
                            nc.vector.tensor_mul(P[:, co + lo:co + lo + 128],
                                                 P[:, co + lo:co + lo + 128], mask01_sb)
                    Ps.append((j, P, lo))
                    if j in pop_js and fq:
                        pop_filler(fq)
                return Ps

            def av_pass(p, qc, Ps):
                """K=128 single-tile AV accumulating both key halves straight
                into one psy tile; then normalize into yT."""
                hE, hO = 2 * p, 2 * p + 1
                q0 = qc * W
                jmax = 4 * qc + 3
                psy = ps_y.tile([HD + 1, 1024], F32, name=f"psy_{p}_{qc}", tag="psy")
                for (j, P, lo) in Ps:
                    for hh, co in ((hE, 0), (hO, W)):
                        nc.tensor.matmul(
                            psy[:, co + lo:co + W],
                            vAug[j][:, hh * (HD + 1):(hh + 1) * (HD + 1)],
                            P[:, co + lo:co + W],
                            start=(j == 0), stop=(j == jmax))
                # normalize: denominators are psy row HD. Stage the row in
                # SBUF f32 (approx_fast reads garbage from PSUM), then the
                # ~5x-faster Newton-seed reciprocal.
                den = pool_n.tile([1, 1024], F32, name=f"den_{p}_{qc}", tag="den")
                nc.vector.tensor_copy(den, psy[HD:HD + 1, :])
                r32 = pool_n.tile([1, 1024], F32, name=f"r32_{p}_{qc}", tag="r32")
                nc.vector.reciprocal_approx_fast(out=r32, in_=den)
                rb = pool_n.tile([HD, 1024], F32, name=f"rb_{p}_{qc}", tag="rb")
                nc.gpsimd.partition_broadcast(rb, r32)
                for so, co in ((0, 0), (64, W)):
                    nc.vector.tensor_mul(yT_sb[p][so:so + HD, q0:q0 + W],
                                         psy[0:HD, co:co + W], rb[:, co:co + W])

            # ============ emission ============
            def body():
                for q in p1_quanta(0):
                    q()
                fq = deque()
                p3_done = []  # p3 quanta emitted
                for qc in range(4):
                    # correctness: any p1(qc) quanta still queued MUST be
                    # emitted before this wave's scores read t-chunk qc
                    while fq and fq[0][0] == ("p1", qc):
                        fq.popleft()[1]()
                    if qc < 3:
                        fq.extend((("p1", qc + 1), q) for q in p1_quanta(qc + 1))
                    if qc >= 1:
                        for tt in range(4 * (qc - 1), 4 * qc):
                            for nch in range(2):
                                fq.append((("p3",), p3_quantum(tt, nch)))
                                p3_done.append((tt, nch))
                    # per-unit filler budget: early waves consume only their
                    # p1 deadline supply so p3 quanta roll into the ACT-bound
                    # wave 3 (which otherwise starves at 8 quanta for ~19
                    # quanta of PE-idle)
                    npop = {0: 2, 1: 2, 2: 2, 3: 3}[qc]
                    for p in range(4):
                        Ps = scores_pass(p, qc, fq, npop)
                        if fq:  # filler bridges the last exp's tail before AV
                            pop_filler(fq)
                        av_pass(p, qc, Ps)
                # drain remaining fillers, then the p3 tail
                for _, q in fq:
                    q()
                for tt in range(TT):
                    for nch in range(2):
                        if (tt, nch) not in p3_done:
                            p3_quantum(tt, nch)()

            if bench_loops:
                with tc.For_i(0, bench_loops, 1):
                    body()
            else:
                body()
    nc.finalize()
    return nc


def _get_nc(bench_loops=None, phases=(1, 2, 3)):
    key = ("nc2", bench_loops)
    if key not in _CACHE:
        _CACHE[key] = _build(bench_loops)
    return _CACHE[key]


def make_in_maps(x, Wk, bk, Wq, bq, Wv, bv, Wp, bp):
    x = np.asarray(x, dtype=np.float32)
    Wk, Wq, Wv, Wp = (np.asarray(a, dtype=np.float32) for a in (Wk, Wq, Wv, Wp))
    bk, bq, bv, bp = (np.asarray(a, dtype=np.float32) for a in (bk, bq, bv, bp))

    mask01 = np.where(np.tril(np.ones((128, 128), dtype=bool)).T, 1.0, 0.0).astype(NP_BF16)
    xT_b = [np.ascontiguousarray(x[b].T).astype(NP_BF16) for b in range(B)]
    in_maps = []
    for c in range(N_CORES):
        b, half = c // 2, c % 2
        hs = half * MPC
        in_maps.append({
            "xT": xT_b[b],
            "wqT": np.ascontiguousarray(Wq[hs:hs + MPC, :].T).astype(NP_BF16),
            "wkT": np.ascontiguousarray(Wk[hs:hs + MPC, :].T).astype(NP_BF16),
            "wvT": np.ascontiguousarray(Wv[hs:hs + MPC, :].T).astype(NP_BF16),
            "wpT": np.ascontiguousarray(Wp[:, hs:hs + MPC].T).astype(NP_BF16),
            "bqc": np.ascontiguousarray(bq[hs:hs + MPC].reshape(MT, 128).T).astype(np.float32),
            "bkc": np.ascontiguousarray(bk[hs:hs + MPC].reshape(MT, 128).T).astype(np.float32),
            "bv": bv[hs:hs + MPC].reshape(1, MPC).astype(np.float32),
            "bp": (bp if half == 0 else np.zeros_like(bp)).reshape(1, C).astype(np.float32),
            "mask01": mask01,
        })
    return in_maps


def kernel(x, Wk, bk, Wq, bq, Wv, bv, Wp, bp, **run_kwargs):
    in_maps = make_in_maps(x, Wk, bk, Wq, bq, Wv, bv, Wp, bp)
    nc = _get_nc()
    res = run_bass_kernel_spmd(nc, in_maps, core_ids=list(range(N_CORES)), **run_kwargs)
    out = np.empty((B, T, C), dtype=np.float32)
    for b in range(B):
        out[b] = (res.results[2 * b]["out"].astype(np.float32)
                  + res.results[2 * b + 1]["out"].astype(np.float32))
    if run_kwargs:
        kernel.last_results = res
    return out


# revision 5
# speedup vs baseline: 1.0428x; 1.0428x over previous
"""Causal self-attention (B=4, T=2048, C=1024, H=16) on 8 NeuronCores — v2.

Same sharding/layout as v1 (core c: batch c//2, head-half c%2; Megatron
column/row parallel; bf16 matmuls, fp32 PSUM; host fp32 sum of the two
per-batch partials).

v2 restructures the schedule around measured engine rates:
  PE   ~296-338 ns per 512-col MM (dual 64-row tiles stream concurrently),
  ACT  (N+352)/1.2 ns + ~150 dispatch (exp, dtype-independent, no fast mode),
  DVE  1x for f32/PSUM tensor_tensor, 2x/4x for bf16 tensor_scalar/copy.
The v1 per-j chain scores->exp->AV made phase 2 ACT-bound (~163us serial).
v2 changes:
  * ACT does ONLY the phase-2 exp. q/k bias evacuations moved to DVE
    tensor_scalar (per-partition bias column), v/p3 evacuations stay DVE.
  * Phase 2 per (pair, qc): scores pass (64-row dual tiles) with "filler"
    PE quanta (phase-1/phase-3 matmul groups) interleaved after every odd
    j, so the PE works while ACT streams exp; then a K=128 single-tile AV
    pass accumulating both key halves into ONE psy tile (kills the v1
    psyA/psyB DVE merge), then normalization (reciprocal_approx_fast +
    gpsimd broadcast + 2 muls).
  * PSUM: scores 2x[128,1024] (4 banks) + p1/p3 groups 2x[128,512]
    (2 banks) + psy [65,1024] (2 banks) = 8 banks; the filler groups have
    their own banks so they never compete with the scores/exp pipeline.
  * Filler supply: p1(tch=qc+1) groups + p3 groups for chunks already
    normalized feed waves qc=0..3; the p3 tail and next iteration's
    p1(tch=0) fill the inter-iteration boundary.
"""

import sys
import types

import numpy as np
from contextlib import ExitStack
from collections import deque

import ml_dtypes

import concourse.bass as bass
import concourse.mybir as mybir
import concourse.tile as tile
from concourse import bacc
from concourse.bass_utils import run_bass_kernel_spmd

try:  # pragma: no cover
    import antenv.axon_hooks  # noqa: F401
except ImportError:  # pragma: no cover
    import antenv

    _stub = types.ModuleType("antenv.axon_hooks")
    _stub.get_axon_ntff_profile_hook = lambda: None
    sys.modules["antenv.axon_hooks"] = _stub
    antenv.axon_hooks = _stub

F32 = mybir.dt.float32
BF16 = mybir.dt.bfloat16
FP8 = mybir.dt.float8e4
DR = mybir.MatmulPerfMode.DoubleRow
EXP = mybir.ActivationFunctionType.Exp
NP_BF16 = ml_dtypes.bfloat16

B, T, C, H = 4, 2048, 1024, 16
HD = C // H              # 64 head dim
N_CORES = 8
HPC = H // 2             # 8 heads per core
MPC = C // 2             # 512 channels per core
MT = MPC // 128          # 4 m-tiles per core
CT = C // 128            # 8 contraction tiles
TC = T // 512            # 4 t-chunks
TT = T // 128            # 16 t-tiles
SCALE = float(1.0 / np.sqrt(HD))
W = 512                  # query chunk width

_CACHE = {}


def _build(bench_loops=None):
    import contextlib

    nc = bacc.Bacc()
    xT = nc.declare_dram_parameter("xT", [C, T], BF16, isOutput=False)
    wqT = nc.declare_dram_parameter("wqT", [C, MPC], BF16, isOutput=False)
    wkT = nc.declare_dram_parameter("wkT", [C, MPC], BF16, isOutput=False)
    wvT = nc.declare_dram_parameter("wvT", [C, MPC], BF16, isOutput=False)
    wpT = nc.declare_dram_parameter("wpT", [MPC, C], BF16, isOutput=False)
    bqc = nc.declare_dram_parameter("bqc", [128, MT], F32, isOutput=False)
    bkc = nc.declare_dram_parameter("bkc", [128, MT], F32, isOutput=False)
    bv = nc.declare_dram_parameter("bv", [1, MPC], F32, isOutput=False)
    bp = nc.declare_dram_parameter("bp", [1, C], F32, isOutput=False)
    mask01d = nc.declare_dram_parameter("mask01", [128, 128], BF16, isOutput=False)
    outp = nc.declare_dram_parameter("out", [T, C], BF16, isOutput=True)

    with tile.TileContext(nc) as tc:
        with ExitStack() as ctx:
            persist = ctx.enter_context(tc.tile_pool(name="persist", bufs=1))
            pool_P = ctx.enter_context(tc.tile_pool(name="pool_P", bufs=1))
            # PSUM budget (8 banks): scores 2x[128,1024]=4, p1/p3 groups
            # 2x[128,512]=2, psy [65,1024]=2.
            ps_sc = ctx.enter_context(tc.tile_pool(name="ps_sc", bufs=2, space="PSUM"))
            ps_mm = ctx.enter_context(tc.tile_pool(name="ps_mm", bufs=2, space="PSUM"))
            ps_y = ctx.enter_context(tc.tile_pool(name="ps_y", bufs=1, space="PSUM"))

            # ---- constants / small tensors ----
            mask01_sb = persist.tile([128, 128], BF16, name="mask01_sb")
            nc.sync.dma_start(out=mask01_sb, in_=mask01d[:, :])
            bqc_sb = persist.tile([128, MT], F32, name="bqc_sb")
            nc.sync.dma_start(out=bqc_sb, in_=bqc[:, :])
            bkc_sb = persist.tile([128, MT], F32, name="bkc_sb")
            nc.sync.dma_start(out=bkc_sb, in_=bkc[:, :])
            bv_sb = persist.tile([1, MPC], F32, name="bv_sb")
            nc.sync.dma_start(out=bv_sb, in_=bv[:, :])
            bp_sb = persist.tile([1, C], F32, name="bp_sb")
            nc.sync.dma_start(out=bp_sb, in_=bp[:, :])
            ones8 = persist.tile([128, 8], BF16, name="ones8")
            nc.vector.memset(ones8, 1.0)
            neg2 = persist.tile([128, 1], F32, name="neg2")
            nc.vector.memset(neg2, -4.5)
            bv_bc = persist.tile([128, MPC], F32, name="bv_bc")
            nc.gpsimd.partition_broadcast(bv_bc, bv_sb)
            bp_bc = persist.tile([128, C], F32, name="bp_bc")
            nc.gpsimd.partition_broadcast(bp_bc, bp_sb)

            # ---- persistent activations ----
            qT_sb = [persist.tile([128, T], BF16, name=f"qT{m}") for m in range(MT)]
            kT_sb = [persist.tile([128, T], BF16, name=f"kT{m}") for m in range(MT)]
            # fp8 vAug PAIRS for DoubleRow AV: tile tp holds key-tiles
            # (2tp, 2tp+1) in column halves 0:520 / 1024:1544 (1024-byte
            # half stride keeps the DoubleRow Ko step 16B-aligned)
            vAug8 = [persist.tile([128, 2048], FP8, name=f"vAug8{t}") for t in range(TT // 2)]
            # bf16 copies of key-tiles 0,1: rows with <64 keys (flush-NaN
            # risk under the -4.5 fp8 shift) live entirely in (qc0, keys
            # 0-127) — that pair runs a bf16 AV path
            vAug_b = [persist.tile([128, HPC * (HD + 1)], BF16, name=f"vAugb{t}")
                      for t in range(2)]
            for t_ in range(2):
                vb = vAug_b[t_].rearrange("p (h w) -> p h w", w=HD + 1)
                nc.vector.tensor_copy(vb[:, :, HD], ones8)
            yT_sb = [persist.tile([128, T], BF16, name=f"yT{m}") for m in range(MT)]

            # constant ones-column of each vAug half, written once (1.0 is
            # exact in fp8)
            for tp in range(TT // 2):
                for i in range(2):
                    va = vAug8[tp][:, i * 1024:i * 1024 + HPC * (HD + 1)].rearrange(
                        "p (h w) -> p h w", w=HD + 1)
                    nc.vector.tensor_copy(va[:, :, HD], ones8)

            # ---- weights in SBUF (loaded once) ----
            pool_w = ctx.enter_context(tc.tile_pool(name="pool_w", bufs=1))
            wq_t = [pool_w.tile([128, MPC], BF16, name=f"wq{c}") for c in range(CT)]
            wk_t = [pool_w.tile([128, MPC], BF16, name=f"wk{c}") for c in range(CT)]
            wv_t = [pool_w.tile([128, MPC], BF16, name=f"wv{c}") for c in range(CT)]
            for c in range(CT):
                nc.sync.dma_start(out=wq_t[c], in_=wqT[c * 128:(c + 1) * 128, :])
                nc.sync.dma_start(out=wk_t[c], in_=wkT[c * 128:(c + 1) * 128, :])
                nc.sync.dma_start(out=wv_t[c], in_=wvT[c * 128:(c + 1) * 128, :])
            wp_t = [pool_w.tile([128, C], BF16, name=f"wp{m}") for m in range(MT)]
            for m in range(MT):
                nc.sync.dma_start(out=wp_t[m], in_=wpT[m * 128:(m + 1) * 128, :])

            pool_xs = ctx.enter_context(tc.tile_pool(name="pool_xs", bufs=1))
            pool_n = ctx.enter_context(tc.tile_pool(name="pool_n", bufs=2))
            pool_o = ctx.enter_context(tc.tile_pool(name="pool_o", bufs=1))

            ADD = mybir.AluOpType.add

            # ============ phase-1 quanta (one matmul group each) ============
            def p1_quanta(tch):
                """12 closures: 4 q-groups, 4 k-groups, 4 v-groups for t-chunk
                tch. The first closure also issues the xs DMAs."""
                t0 = tch * 512
                xs = []

                def dma():
                    for c in range(CT):
                        x_ = pool_xs.tile([128, 512], BF16, name=f"xs_{tch}_{c}",
                                          tag="xs", bufs=16)
                        nc.sync.dma_start(out=x_, in_=xT[c * 128:(c + 1) * 128, t0:t0 + 512])
                        xs.append(x_)

                def qk_group(wt, bias_col, dst, m, first):
                    def run():
                        if first:
                            dma()
                        ps = ps_mm.tile([128, 512], F32, name=f"p1_{tch}_{id(wt)}_{m}",
                                        tag="psmm")
                        for c in range(CT):
                            nc.tensor.matmul(ps, wt[c][:, m * 128:(m + 1) * 128], xs[c],
                                             start=(c == 0), stop=(c == CT - 1))
                        nc.vector.tensor_scalar(out=dst[m][:, t0:t0 + 512], in0=ps,
                                                scalar1=bias_col[:, m:m + 1],
                                                scalar2=None, op0=ADD)
                    return run

                def v_group(tt):
                    def run():
                        tg = tch * 4 + tt
                        ps = ps_mm.tile([128, MPC], F32, name=f"psv_{tg}", tag="psmm")
                        for c in range(CT):
                            nc.tensor.matmul(ps, xs[c][:, tt * 128:(tt + 1) * 128], wv_t[c],
                                             start=(c == 0), stop=(c == CT - 1))
                        i = tg % 2
                        va = vAug8[tg // 2][:, i * 1024:i * 1024 + HPC * (HD + 1)].rearrange(
                            "p (h w) -> p h w", w=HD + 1)
                        nc.vector.tensor_add(va[:, :, 0:HD],
                                             ps.rearrange("p (h w) -> p h w", w=HD),
                                             bv_bc.rearrange("p (h w) -> p h w", w=HD))
                        if tg < 2:  # bf16 copy for the qc0 bf16 AV path
                            vb = vAug_b[tg].rearrange("p (h w) -> p h w", w=HD + 1)
                            nc.vector.tensor_add(vb[:, :, 0:HD],
                                                 ps.rearrange("p (h w) -> p h w", w=HD),
                                                 bv_bc.rearrange("p (h w) -> p h w", w=HD))
                    return run

                out = []
                for m in range(MT):
                    out.append(qk_group(wq_t, bqc_sb, qT_sb, m, first=(m == 0)))
                for m in range(MT):
                    out.append(qk_group(wk_t, bkc_sb, kT_sb, m, first=False))
                for tt in range(4):
                    out.append(v_group(tt))
                return out

            # ============ phase-3 quanta ============
            def p3_quantum(tt, nch):
                def run():
                    n0 = nch * 512
                    ps = ps_mm.tile([128, 512], F32, name=f"pso_{tt}_{nch}", tag="psmm")
                    for m in range(MT):
                        nc.tensor.matmul(ps, yT_sb[m][:, tt * 128:(tt + 1) * 128],
                                         wp_t[m][:, n0:n0 + 512],
                                         start=(m == 0), stop=(m == MT - 1))
                    o_sb = pool_o.tile([128, 512], BF16, name=f"o_{tt}_{nch}", tag="o", bufs=3)
                    nc.vector.tensor_add(o_sb, ps, bp_bc[:, n0:n0 + 512])
                    nc.sync.dma_start(out=outp[tt * 128:(tt + 1) * 128, n0:n0 + 512], in_=o_sb)
                return run

            def pop_filler(fq):
                """Emit one filler, preferring deadline-bound p1 quanta over
                p3 quanta that rolled over from earlier waves."""
                for i, (kind, q) in enumerate(fq):
                    if kind[0] == "p1":
                        del fq[i]
                        q()
                        return
                fq.popleft()[1]()

            # ============ phase-2 passes ============
            def scores_pass(p, qc, fq, npop):
                """64-row dual-tile score MMs + exp ACT + diagonal mask, with
                up to npop filler quanta spread over the odd j's. Returns P
                tiles."""
                hE, hO = 2 * p, 2 * p + 1
                kT_h, qT_h = kT_sb[p], qT_sb[p]
                q0 = qc * W
                jmax = 4 * qc + 3
                odd = list(range(1, jmax + 1, 2))
                pop_js = {odd[min(len(odd) - 1, (i + 1) * len(odd) // (npop + 1))]
                          for i in range(npop)}
                Ps = []
                P2 = None
                prev_lo = 0
                for j in range(jmax + 1):
                    lo = max(0, 128 * j - q0)
                    pss = ps_sc.tile([128, 1024], F32, name=f"pss_{p}_{qc}_{j}", tag="pssc")
                    for half, co in ((0, 0), (64, W)):
                        nc.tensor.matmul(
                            pss[:, co + lo:co + W],
                            kT_h[half:half + HD, j * 128:(j + 1) * 128],
                            qT_h[half:half + HD, q0 + lo:q0 + W],
                            start=True, stop=True, tile_position=(half, 0))
                    pv = pss.rearrange("p (g w) -> p g w", w=W)
                    if qc == 0 and j < 2:
                        # bf16 P for the flush-risk pair (keys 0-255); same
                        # -4.5 shift keeps the scale consistent with fp8
                        Pb = pool_P.tile([128, 1024], BF16, name=f"Pb_{p}_{j}",
                                         tag="Pb", bufs=3)
                        Pv = Pb.rearrange("p (g w) -> p g w", w=W)
                        nc.scalar.activation(out=Pv[:, :, lo:W], in_=pv[:, :, lo:W],
                                             func=EXP, scale=SCALE, bias=neg2[:, 0:1])
                        for co in (0, W):  # j=0,1 are both diagonal in qc0
                            nc.vector.tensor_mul(Pb[:, co + lo:co + lo + 128],
                                                 Pb[:, co + lo:co + lo + 128],
                                                 mask01_sb)
                        Ps.append(("b", j, Pb, lo))
                        prev_lo = lo
                        if j in pop_js and fq:
                            pop_filler(fq)
                        continue
                    if j % 2 == 0:
                        # fp8 P PAIR tile: j even -> cols 0:1024, odd ->
                        # 1024:2048 (the DoubleRow [128,2,F] interleave view)
                        P2 = pool_P.tile([128, 2048], FP8, name=f"P2_{p}_{qc}_{j}",
                                         tag="P", bufs=10)
                    half_v = P2[:, (j % 2) * 1024:(j % 2 + 1) * 1024]
                    Pv = half_v.rearrange("p (g w) -> p g w", w=W)
                    # exp(s - 4.5): softmax-shift keeps the ratio exact while
                    # capping the max below fp8e4's 240 (raw exp can hit 8051)
                    nc.scalar.activation(out=Pv[:, :, lo:W], in_=pv[:, :, lo:W],
                                         func=EXP, scale=SCALE, bias=neg2[:, 0:1])
                    if 128 * j >= q0:  # diagonal block: in-tile causal mask
                        for co in (0, W):
                            nc.vector.tensor_mul(half_v[:, co + lo:co + lo + 128],
                                                 half_v[:, co + lo:co + lo + 128],
                                                 mask01_sb)
                    if j % 2 == 1:
                        if lo > prev_lo:
                            # odd block's columns below its diagonal were not
                            # written by the exp; the pair-MM reads them
                            nc.vector.memset(Pv[:, :, prev_lo:lo], 0.0)
                        Ps.append(("f", j // 2, P2, prev_lo))
                    prev_lo = lo
                    if j in pop_js and fq:
                        pop_filler(fq)
                return Ps

            def av_pass(p, qc, Ps):
                """K=128 single-tile AV accumulating both key halves straight
                into one psy tile; then normalize into yT."""
                hE, hO = 2 * p, 2 * p + 1
                q0 = qc * W
                jmax = 4 * qc + 3
                psy = ps_y.tile([HD + 1, 1024], F32, name=f"psy_{p}_{qc}", tag="psy")
                for idx, ent in enumerate(Ps):
                    first, last = idx == 0, idx == len(Ps) - 1
                    if ent[0] == "b":  # bf16 K=128 path (qc0 keys 0-255)
                        _, j, Pb, lo = ent
                        for hh, co in ((hE, 0), (hO, W)):
                            nc.tensor.matmul(
                                psy[:, co + lo:co + W],
                                vAug_b[j][:, hh * (HD + 1):(hh + 1) * (HD + 1)],
                                Pb[:, co + lo:co + W],
                                start=first, stop=last)
                    else:
                        _, jp, P2, lo = ent
                        lv = vAug8[jp].rearrange("p (i x) -> p i x", x=1024)
                        rv = P2.rearrange("p (i x) -> p i x", x=1024)
                        for hh, co in ((hE, 0), (hO, W)):
                            nc.tensor.matmul(
                                psy[:, co + lo:co + W],
                                lv[:, :, hh * (HD + 1):(hh + 1) * (HD + 1)],
                                rv[:, :, co + lo:co + W],
                                start=first, stop=last, perf_mode=DR)
                # normalize: denominators are psy row HD. Stage the row in
                # SBUF f32 (approx_fast reads garbage from PSUM), then the
                # ~5x-faster Newton-seed reciprocal.
                den = pool_n.tile([1, 1024], F32, name=f"den_{p}_{qc}", tag="den")
                nc.vector.tensor_copy(den, psy[HD:HD + 1, :])
                r32 = pool_n.tile([1, 1024], F32, name=f"r32_{p}_{qc}", tag="r32")
                nc.vector.reciprocal_approx_fast(out=r32, in_=den)
                rb = pool_n.tile([HD, 1024], F32, name=f"rb_{p}_{qc}", tag="rb")
                nc.gpsimd.partition_broadcast(rb, r32)
                for so, co in ((0, 0), (64, W)):
                    nc.vector.tensor_mul(yT_sb[p][so:so + HD, q0:q0 + W],
                                         psy[0:HD, co:co + W], rb[:, co:co + W])

            # ============ emission ============
            def body():
                for q in p1_quanta(0):
                    q()
                fq = deque()
                p3_done = []  # p3 quanta emitted
                for qc in range(4):
                    # correctness: any p1(qc) quanta still queued MUST be
                    # emitted before this wave's scores read t-chunk qc
                    while fq and fq[0][0] == ("p1", qc):
                        fq.popleft()[1]()
                    if qc < 3:
                        fq.extend((("p1", qc + 1), q) for q in p1_quanta(qc + 1))
                    if qc >= 1:
                        for tt in range(4 * (qc - 1), 4 * qc):
                            for nch in range(2):
                                fq.append((("p3",), p3_quantum(tt, nch)))
                                p3_done.append((tt, nch))
                    # per-unit filler budget: early waves consume only their
                    # p1 deadline supply so p3 quanta roll into the ACT-bound
                    # wave 3 (which otherwise starves at 8 quanta for ~19
                    # quanta of PE-idle)
                    npop = {0: 2, 1: 2, 2: 2, 3: 4}[qc]
                    for p in range(4):
                        Ps = scores_pass(p, qc, fq, npop)
                        if fq:  # filler bridges the last exp's tail before AV
                            pop_filler(fq)
                        av_pass(p, qc, Ps)
                # drain remaining fillers, then the p3 tail
                for _, q in fq:
                    q()
                for tt in range(TT):
                    for nch in range(2):
                        if (tt, nch) not in p3_done:
                            p3_quantum(tt, nch)()

            if bench_loops:
                with tc.For_i(0, bench_loops, 1):
                    body()
            else:
                body()
    nc.finalize()
    return nc


def _get_nc(bench_loops=None, phases=(1, 2, 3)):
    key = ("nc2", bench_loops)
    if key not in _CACHE:
        _CACHE[key] = _build(bench_loops)
    return _CACHE[key]


def make_in_maps(x, Wk, bk, Wq, bq, Wv, bv, Wp, bp):
    x = np.asarray(x, dtype=np.float32)
    Wk, Wq, Wv, Wp = (np.asarray(a, dtype=np.float32) for a in (Wk, Wq, Wv, Wp))
    bk, bq, bv, bp = (np.asarray(a, dtype=np.float32) for a in (bk, bq, bv, bp))

    mask01 = np.where(np.tril(np.ones((128, 128), dtype=bool)).T, 1.0, 0.0).astype(NP_BF16)
    xT_b = [np.ascontiguousarray(x[b].T).astype(NP_BF16) for b in range(B)]
    in_maps = []
    for c in range(N_CORES):
        b, half = c // 2, c % 2
        hs = half * MPC
        in_maps.append({
            "xT": xT_b[b],
            "wqT": np.ascontiguousarray(Wq[hs:hs + MPC, :].T).astype(NP_BF16),
            "wkT": np.ascontiguousarray(Wk[hs:hs + MPC, :].T).astype(NP_BF16),
            "wvT": np.ascontiguousarray(Wv[hs:hs + MPC, :].T).astype(NP_BF16),
            "wpT": np.ascontiguousarray(Wp[:, hs:hs + MPC].T).astype(NP_BF16),
            "bqc": np.ascontiguousarray(bq[hs:hs + MPC].reshape(MT, 128).T).astype(np.float32),
            "bkc": np.ascontiguousarray(bk[hs:hs + MPC].reshape(MT, 128).T).astype(np.float32),
            "bv": bv[hs:hs + MPC].reshape(1, MPC).astype(np.float32),
            "bp": (bp if half == 0 else np.zeros_like(bp)).reshape(1, C).astype(np.float32),
            "mask01": mask01,
        })
    return in_maps


def kernel(x, Wk, bk, Wq, bq, Wv, bv, Wp, bp, **run_kwargs):
    in_maps = make_in_maps(x, Wk, bk, Wq, bq, Wv, bv, Wp, bp)
    nc = _get_nc()
    res = run_bass_kernel_spmd(nc, in_maps, core_ids=list(range(N_CORES)), **run_kwargs)
    out = np.empty((B, T, C), dtype=np.float32)
    for b in range(B):
        out[b] = (res.results[2 * b]["out"].astype(np.float32)
                  + res.results[2 * b + 1]["out"].astype(np.float32))
    if run_kwargs:
        kernel.last_results = res
    return out
